# revision 18
# baseline (speedup 1.0000x reference)
"""Trainium2 Bass kernel for nn_BatchFFTMA: 9216 independent 65x65 FFT-MA sims.

Math (validated against the jax reference; see v1 docstring for the
derivation of the phase folds):
  For each window w (patch p = noise[r0:r0+65, c0:c0+65], angle theta):
    Wf' = Cpt^T p Cpt   with Cpt = F*diag((-1)^k)
    E   = Re(Cq^T R Cq) with Cq = F*diag(e^{-2pi i 33 k/65})
    R   = exp(-sqrt(q)), q = alpha*x_r^2 + beta*x_c^2 + gamma*x_r*x_c
    gp  = sqrt(relu(E+1e-8)), gn = sqrt(relu(-(E+1e-8)))
    Xr  = Wf'_r*gp*SGP - Wf'_i*gn*SGN   (SGP/SGN: +-1 branch-cut fields)
    VC = sum(Xr); X00 = Xr[0,0]; S = sum((Wf'_r^2+Wf'_i^2)*|E+1e-8|)
    out_w = ((VC-X00)/N^2) / (sqrt((S-X00^2)/(N^2(N^2-1))) + 1e-6)

v2 compute structure (per core, 1152 windows, CB=18 windows/chunk):
  stage1 per window (lhsT = data):  o1p = P^T [Cr33|Ci33] (bf16),
                                    o1r = R^T [Qr33|Qi33] (f32)
  stage2 batched (lhsT = consts):   wfT_re/im = Cr^T o1pA -/+ ..., (bf16)
                                    E^T = Qr^T o1rA - Qi^T o1rB   (f32r)
  -> spectra come out transposed; all downstream sums are transpose-
  invariant. The spectral chain runs on halved columns k1=0..32 with x2
  weights folded into the SGP/SGN constants: the fields are Hermitian-even
  under joint index negation except on the k1+k2=65 line, where the E<0
  branch makes Xr odd (net zero) -> SGN weights are zeroed there.
  sqrt/exp phases batch over SC=4 chunks so the ACT table set switches only
  twice per superchunk (no table fits both sqrt and exp).

I/O strategy (axon tunnel: ~50-80ms fixed RTT + ~11ms/MB, so payload rules):
  constants committed to device once at runner build; per-call payload per
  core = noise rows [76,160] bf16 (24KB) + alpha/beta/gamma [3,1152] f32
  (14KB); final normalize on device, output [1,1152] f32 per core.
"""
import numpy as np
import ml_dtypes

H, W, D = 96, 96, 32
N = 65
N2 = N * N
A_, B_ = 15.0, 3.0
NCORE = 8
WPC = H * W // NCORE      # 1152 windows per core
RPC = H // NCORE          # 12 output rows per core
CB = 18                   # windows per chunk
GRP = 6                   # windows per stage1 matmul/PSUM group
NGRP = CB // GRP
BW = 9                    # windows per stage2 block
NH = 34                   # halved spectral columns (even, incl. both of the 32/33 mirror pair)
SC = 4                    # chunks per superchunk (ACT table batching)

_bf16 = ml_dtypes.bfloat16


def _host_constants():
    k = np.arange(N)
    F = np.exp(-2j * np.pi * np.outer(k, k) / N)
    Cpt = F * ((-1.0) ** k)[None, :]
    Cq = F * np.exp(-2j * np.pi * k * 33 / N)[None, :]
    Cr = Cpt.real.astype(np.float32)
    Ci = Cpt.imag.astype(np.float32)
    Qr = Cq.real.astype(np.float32)
    Qi = Cq.imag.astype(np.float32)
    pconst = np.concatenate(
        [Cr[:, :NH], Ci[:, :NH], Cr, Ci, -Ci], axis=1
    ).astype(_bf16)                                                # [65, 261]
    rconst = np.concatenate(
        [Qr[:, :NH], Qi[:, :NH], Qr, -Qi], axis=1
    ).astype(np.float32)                                           # [65, 196]

    k1, k2 = np.meshgrid(k, k, indexing="ij")
    ksum = k1 + k2
    m = ksum % N
    extra = (-1.0) ** (ksum // N)
    SGP = extra * np.where(m <= 32, 1.0, -1.0)
    SGN = extra * np.where(m == 0, 1.0, -1.0)
    wgt = np.ones(NH); wgt[1:32] = 2.0   # n=0,32,33 -> 1; n=1..31 -> 2
    SGP_h = SGP[:, :NH] * wgt[None, :]
    SGN_h = SGN[:, :NH] * wgt[None, :]
    for n in range(1, 32):
        SGN_h[(N - n) % N, n] = 0.0   # k1+k2=65 line: odd, net zero (mirror absent)
    sgp_t = np.tile(SGP_h, (1, CB)).astype(_bf16)   # [65, NH*CB]
    sgn_t = np.tile(SGN_h, (1, CB)).astype(_bf16)
    sgconst = np.concatenate([sgp_t, sgn_t], axis=1)  # [65, 2*NH*CB]
    wgt33 = np.tile(wgt[None, :], (N, 1)).astype(_bf16)  # [65, 33]

    x = np.linspace(-D, D, N, dtype=np.float32)
    x2 = x * x
    ones = np.ones(N, np.float32)
    # q[r, (w,c)] = x2[r]*rq[0] + 1*rq[1] + x[r]*rq[2]
    # rq[h, (w,c)] = abg[h, w] * basis2[h, c]
    qbasis = np.stack([x2, ones, x]).astype(np.float32)   # [3, 65]
    basis2 = np.stack([ones, x2, x]).astype(np.float32)   # [3, 65]
    ones65 = np.ones((N, 1), np.float32)
    return pconst, rconst, sgconst, wgt33, qbasis, basis2, ones65


def _build_program(nchunk):
    import concourse.bacc as bacc
    import concourse.mybir as mybir
    from concourse.tile import TileContext

    f32 = mybir.dt.float32
    f32r = mybir.dt.float32r
    f16 = mybir.dt.float16
    bf16 = mybir.dt.bfloat16
    AF = mybir.ActivationFunctionType
    ALU = mybir.AluOpType
    AX = mybir.AxisListType

    nwin = nchunk * CB
    assert nchunk % SC == 0

    nc = bacc.Bacc()
    nz_in = nc.declare_dram_parameter("nz", [RPC + N - 1, 160], bf16, isOutput=False)
    abg_in = nc.declare_dram_parameter("abg", [3, nwin], f16, isOutput=False)
    W2 = 2 * NH
    pconst_in = nc.declare_dram_parameter("pconst", [N, W2 + 195], bf16, isOutput=False)
    rconst_in = nc.declare_dram_parameter("rconst", [N, W2 + 130], f32, isOutput=False)
    sg_in = nc.declare_dram_parameter("sgconst", [N, 2 * NH * CB], bf16, isOutput=False)
    wgt_in = nc.declare_dram_parameter("wgt33", [N, NH], bf16, isOutput=False)
    qbasis_in = nc.declare_dram_parameter("qbasis", [3, N], f32, isOutput=False)
    basis2_in = nc.declare_dram_parameter("basis2", [3, N], f32, isOutput=False)
    ones_in = nc.declare_dram_parameter("ones65", [N, 1], f32, isOutput=False)
    out_d = nc.declare_dram_parameter("out", [1, nwin], f32, isOutput=True)

    with TileContext(nc) as tc:
        with (
            tc.tile_pool(name="const", bufs=1) as cpool,
            tc.tile_pool(name="qsb", bufs=2) as qpool,
            tc.tile_pool(name="rqsb", bufs=SC + 1) as rqpool,
            tc.tile_pool(name="rsb", bufs=2) as rpool,
            tc.tile_pool(name="o1sb", bufs=2) as o1pool,
            tc.tile_pool(name="wfsb", bufs=2) as wfpool,
            tc.tile_pool(name="spec", bufs=2) as spool,
            tc.tile_pool(name="psq", bufs=2, space="PSUM") as pq_,
            tc.tile_pool(name="ps1p", bufs=1, space="PSUM") as pp1,
            tc.tile_pool(name="ps1r", bufs=1, space="PSUM") as pr1,
            tc.tile_pool(name="psw2", bufs=1, space="PSUM") as pw2,
            tc.tile_pool(name="pse2", bufs=2, space="PSUM") as pe2,
        ):
            pc = cpool.tile([N, W2 + 195], bf16)
            nc.sync.dma_start(out=pc[:], in_=pconst_in[:])
            rc = cpool.tile([N, W2 + 130], f32)
            nc.sync.dma_start(out=rc[:], in_=rconst_in[:])
            sg = cpool.tile([N, 2 * NH * CB], bf16)
            nc.sync.dma_start(out=sg[:], in_=sg_in[:])
            wgt = cpool.tile([N, NH], bf16)
            nc.sync.dma_start(out=wgt[:], in_=wgt_in[:])
            qbasis = cpool.tile([3, N], f32)
            nc.sync.dma_start(out=qbasis[:], in_=qbasis_in[:])
            basis2 = cpool.tile([3, N], f32)
            nc.sync.dma_start(out=basis2[:], in_=basis2_in[:])
            ones65 = cpool.tile([N, 1], f32)
            nc.sync.dma_start(out=ones65[:], in_=ones_in[:])
            abg = cpool.tile([3, nwin], f16)
            nc.sync.dma_start(out=abg[:], in_=abg_in[:])
            # f32r copy of [Qr | -Qi] for the 1-cycle stage2 matmuls (the
            # verifier requires f32r operands to be produced as f32r)
            rcr = cpool.tile([N, 130], f32r, tag="rcr")
            nc.vector.tensor_copy(rcr[:], rc[:, W2:W2 + 130])

            strips = cpool.tile([N, RPC * 160], bf16)
            for i in range(RPC):
                nc.sync.dma_start(
                    out=strips[:, i * 160:(i + 1) * 160], in_=nz_in[i:i + N, :]
                )

            # clock warm-up: tiny reads so ACT/DVE observe each setup DMA
            # queue once, capping per-instruction sync waits downstream.
            warm_s = cpool.tile([1, 9], f32, tag="warm_s")
            warm_v = cpool.tile([1, 9], f32, tag="warm_v")
            for idx, src in enumerate(
                (strips, pc, rc, sg, wgt, qbasis, basis2, ones65, abg)
            ):
                nc.scalar.copy(warm_s[0:1, idx:idx + 1], src[0:1, 0:1])
                nc.vector.tensor_copy(warm_v[0:1, idx:idx + 1], src[0:1, 0:1])

            epsp = cpool.tile([N, 1], f32, tag="epsp")
            nc.gpsimd.memset(epsp[:], 1e-8)
            epsn = cpool.tile([N, 1], f32, tag="epsn")
            nc.gpsimd.memset(epsn[:], -1e-8)

            vcs = cpool.tile([N, nwin], f32)
            ss = cpool.tile([N, nwin], f32)
            mus = cpool.tile([1, nwin], f32)

            CWID = CB * N           # 1170 cols per chunk
            for sc0 in range(0, nchunk, SC):
                # ---- phase A: q fields + sqrt (ACT sqrt-table), SC-wide ----
                s_sb = qpool.tile([N, SC * CWID], f32, tag="s_sb")
                for j in range(SC):
                    cw0 = (sc0 + j) * CB
                    rq_sb = rqpool.tile([3, CB * N], f32, tag="rq")
                    rq3 = rq_sb[:].rearrange("p (w c) -> p w c", c=N)
                    abg_v = abg[:, cw0:cw0 + CB].unsqueeze(2).broadcast_to(
                        [3, CB, N]
                    )
                    b2_v = basis2[:].unsqueeze(1).broadcast_to([3, CB, N])
                    nc.vector.tensor_tensor(rq3, abg_v, b2_v, op=ALU.mult)
                    for h in range(3):
                        qps = pq_.tile([N, 390], f32, tag="qps")
                        nc.tensor.matmul(
                            qps[:], qbasis[:], rq_sb[:, h * 390:(h + 1) * 390],
                            start=True, stop=True,
                        )
                        nc.scalar.activation(
                            s_sb[:, j * CWID + h * 390:j * CWID + (h + 1) * 390],
                            qps[:], AF.Sqrt,
                        )
                # ---- phase B: ONE wide exp per superchunk (2 table loads) ----
                r_sb4 = rpool.tile([N, SC * CWID], f32, tag="r_sb")
                nc.scalar.activation(r_sb4[:], s_sb[:], AF.Exp, scale=-1.0)

                # ---- phase C: DFTs + spectral chain per chunk ----
                for j in range(SC):
                    ch = sc0 + j
                    cw0 = ch * CB
                    r_sb = r_sb4[:, j * CWID:(j + 1) * CWID]

                    o1p = o1pool.tile([N, CB * W2], bf16, tag="o1p")
                    o1r = o1pool.tile([N, CB * W2], f32r, tag="o1r")
                    for g in range(NGRP):
                        ps1p = pp1.tile([N, GRP * W2], f32, tag="ps1p")
                        for t in range(GRP):
                            w = cw0 + g * GRP + t
                            r0, c0 = divmod(w, 96)
                            lhs = strips[:, r0 * 160 + c0: r0 * 160 + c0 + N]
                            nc.tensor.matmul(
                                ps1p[:, t * W2:(t + 1) * W2], lhs, pc[:, 0:W2],
                                start=True, stop=True,
                            )
                        nc.scalar.copy(
                            o1p[:, g * GRP * W2:(g + 1) * GRP * W2], ps1p[:]
                        )
                        ps1r = pr1.tile([N, GRP * W2], f32, tag="ps1r")
                        for t in range(GRP):
                            wl = (g * GRP + t) * N
                            nc.tensor.matmul(
                                ps1r[:, t * W2:(t + 1) * W2],
                                r_sb[:, wl:wl + N], rc[:, 0:W2],
                                start=True, stop=True,
                            )
                        nc.vector.tensor_copy(
                            o1r[:, g * GRP * W2:(g + 1) * GRP * W2], ps1r[:]
                        )

                    # batched stage2 + spectral front-end, per 9-window block
                    wfA = wfpool.tile([N, CB * NH], bf16, tag="wfA")
                    wfB = wfpool.tile([N, CB * NH], bf16, tag="wfB")
                    rp = spool.tile([N, CB * NH], bf16, tag="rp")
                    rn = spool.tile([N, CB * NH], bf16, tag="rn")
                    azt = spool.tile([N, CB * NH], bf16, tag="azt")
                    for b in range(2):
                        o1p3 = o1p[:, b * BW * W2:(b + 1) * BW * W2].rearrange(
                            "p (w t) -> p w t", t=W2
                        )
                        o1r3 = o1r[:, b * BW * W2:(b + 1) * BW * W2].rearrange(
                            "p (w t) -> p w t", t=W2
                        )
                        sl = slice(b * BW * NH, (b + 1) * BW * NH)

                        wre = pw2.tile([N, BW * NH], f32, tag="wre")
                        nc.tensor.matmul(
                            wre[:], pc[:, W2:W2 + 65], o1p3[:, :, 0:NH],
                            start=True, stop=False,
                        )
                        nc.tensor.matmul(
                            wre[:], pc[:, W2 + 130:W2 + 195], o1p3[:, :, NH:W2],
                            start=False, stop=True,
                        )
                        nc.scalar.copy(wfA[:, sl], wre[:])
                        wim = pw2.tile([N, BW * NH], f32, tag="wim")
                        nc.tensor.matmul(
                            wim[:], pc[:, W2 + 65:W2 + 130], o1p3[:, :, 0:NH],
                            start=True, stop=False,
                        )
                        nc.tensor.matmul(
                            wim[:], pc[:, W2:W2 + 65], o1p3[:, :, NH:W2],
                            start=False, stop=True,
                        )
                        nc.scalar.copy(wfB[:, sl], wim[:])

                        e2 = pe2.tile([N, BW * NH], f32, tag="e2")
                        nc.tensor.matmul(
                            e2[:], rcr[:, 0:65], o1r3[:, :, 0:NH],
                            start=True, stop=False,
                        )
                        nc.tensor.matmul(
                            e2[:], rcr[:, 65:130], o1r3[:, :, NH:W2],
                            start=False, stop=True,
                        )
                        # rp = relu(E+eps), rn = relu(-(E+eps)), az = |E+eps|
                        nc.scalar.activation(rp[:, sl], e2[:], AF.Relu, bias=epsp[:])
                        nc.scalar.activation(
                            rn[:, sl], e2[:], AF.Relu, scale=-1.0, bias=epsn[:]
                        )

                    FD = CB * NH
                    # az = |E+eps| = relu(E+eps) + relu(-(E+eps))
                    nc.vector.tensor_add(azt[:], rp[:], rn[:])
                    azw = spool.tile([N, FD], bf16, tag="azw")
                    az3o = azw[:].rearrange("p (w c) -> p w c", c=NH)
                    az3i = azt[:].rearrange("p (w c) -> p w c", c=NH)
                    wgt_v = wgt[:].unsqueeze(1).broadcast_to([N, CB, NH])
                    nc.vector.tensor_tensor(az3o, az3i, wgt_v, op=ALU.mult)

                    gp = spool.tile([N, FD], bf16, tag="gp")
                    nc.scalar.activation(gp[:], rp[:], AF.Sqrt)
                    gn = spool.tile([N, FD], bf16, tag="gn")
                    nc.scalar.activation(gn[:], rn[:], AF.Sqrt)
                    gps = spool.tile([N, FD], bf16, tag="gps")
                    nc.vector.tensor_mul(gps[:], gp[:], sg[:, 0:FD])
                    gns = spool.tile([N, FD], bf16, tag="gns")
                    nc.vector.tensor_mul(gns[:], gn[:], sg[:, FD:2 * FD])
                    a_t = spool.tile([N, FD], bf16, tag="a_t")
                    nc.vector.tensor_mul(a_t[:], wfA[:], gps[:])
                    b_t = spool.tile([N, FD], bf16, tag="b_t")
                    nc.vector.tensor_mul(b_t[:], wfB[:], gns[:])
                    xr = spool.tile([N, FD], bf16, tag="xr")
                    nc.vector.tensor_sub(xr[:], a_t[:], b_t[:])
                    wr2 = spool.tile([N, FD], bf16, tag="wr2")
                    nc.vector.tensor_mul(wr2[:], wfA[:], wfA[:])
                    wi2 = spool.tile([N, FD], bf16, tag="wi2")
                    nc.vector.tensor_mul(wi2[:], wfB[:], wfB[:])
                    w2s = spool.tile([N, FD], bf16, tag="w2s")
                    nc.vector.tensor_add(w2s[:], wr2[:], wi2[:])
                    st = spool.tile([N, FD], bf16, tag="st")
                    nc.vector.tensor_mul(st[:], w2s[:], azw[:])

                    xr3 = xr[:].rearrange("p (w c) -> p w c", c=NH)
                    st3 = st[:].rearrange("p (w c) -> p w c", c=NH)
                    nc.vector.tensor_reduce(
                        vcs[:, cw0:cw0 + CB], xr3, axis=AX.X, op=ALU.add
                    )
                    nc.vector.tensor_reduce(
                        ss[:, cw0:cw0 + CB], st3, axis=AX.X, op=ALU.add
                    )
                    nc.vector.tensor_copy(mus[0:1, cw0:cw0 + CB], xr3[0:1, :, 0])

            # ---- on-device finalize: one scalar per window ----
            vc_row = cpool.tile([1, nwin], f32, tag="vc_row")
            s_row = cpool.tile([1, nwin], f32, tag="s_row")
            t1 = cpool.tile([1, nwin], f32, tag="t1")
            t2 = cpool.tile([1, nwin], f32, tag="t2")
            t3 = cpool.tile([1, nwin], f32, tag="t3")
            NB = 384
            for b0 in range(0, nwin, NB):
                bn = min(NB, nwin - b0)
                pv = pq_.tile([1, NB], f32, tag="qps")
                nc.tensor.matmul(
                    pv[:, 0:bn], ones65[:], vcs[:, b0:b0 + bn],
                    start=True, stop=True,
                )
                nc.vector.tensor_copy(vc_row[0:1, b0:b0 + bn], pv[:, 0:bn])
                pv2 = pq_.tile([1, NB], f32, tag="qps")
                nc.tensor.matmul(
                    pv2[:, 0:bn], ones65[:], ss[:, b0:b0 + bn],
                    start=True, stop=True,
                )
                nc.vector.tensor_copy(s_row[0:1, b0:b0 + bn], pv2[:, 0:bn])

            # out = ((VC-mu)/N2) / (sqrt(max(S-mu^2,0)/(N2*(N2-1))) + 1e-6)
            nc.vector.tensor_sub(t1[:], vc_row[:], mus[:])
            nc.vector.tensor_mul(t2[:], mus[:], mus[:])
            nc.vector.tensor_sub(t3[:], s_row[:], t2[:])
            nc.vector.tensor_scalar_max(t2[:], t3[:], 0.0)
            c2 = 1.0 / (float(N2) * (N2 - 1.0))
            nc.scalar.activation(t3[:], t2[:], AF.Sqrt, scale=c2)
            nc.vector.tensor_scalar_add(t2[:], t3[:], 1e-6)
            nc.vector.reciprocal(t3[:], t2[:])
            nc.vector.scalar_tensor_tensor(
                t2[:], t1[:], 1.0 / N2, t3[:], op0=ALU.mult, op1=ALU.mult
            )
            nc.sync.dma_start(out=out_d[:], in_=t2[:])
    if not nc.is_finalized():
        nc.finalize()
    return nc


_HOST_CONSTS = None


def _host_inputs(angle_matrix, noise, nchunk):
    """Per-core input maps. Core c owns output rows [12c, 12c+12)."""
    global _HOST_CONSTS
    if _HOST_CONSTS is None:
        _HOST_CONSTS = _host_constants()
    pconst, rconst, sgconst, wgt33, qbasis, basis2, ones65 = _HOST_CONSTS
    noise2d = np.asarray(noise, dtype=np.float32)[0, 0]
    nzb = noise2d.astype(_bf16)
    ang = np.asarray(angle_matrix, dtype=np.float32).reshape(-1).astype(np.float64)
    c = np.cos(ang); s = np.sin(ang)
    alpha = (c * c / A_**2 + s * s / B_**2).astype(np.float32)
    beta = (s * s / A_**2 + c * c / B_**2).astype(np.float32)
    gamma = (2 * c * s * (1 / A_**2 - 1 / B_**2)).astype(np.float32)
    abg_all = np.stack([alpha, beta, gamma])  # [3, 9216]

    in_maps = []
    for core in range(NCORE):
        w0 = core * WPC
        r_base = core * RPC
        in_maps.append({
            "nz": np.ascontiguousarray(nzb[r_base:r_base + RPC + N - 1, :]),
            "abg": np.ascontiguousarray(abg_all[:, w0:w0 + WPC]).astype(np.float16),
            "pconst": pconst,
            "rconst": rconst,
            "sgconst": sgconst,
            "wgt33": wgt33,
            "qbasis": qbasis,
            "basis2": basis2,
            "ones65": ones65,
        })
    return in_maps


def _finalize(core_outs):
    """core_outs: list of [1, 1152] arrays -> [96, 96] output."""
    return np.concatenate(
        [arr.reshape(RPC, 96) for arr in core_outs], axis=0
    ).astype(np.float32)


_PROG = {}
_RUNNER = {}
_CONST_NAMES = frozenset(
    {"pconst", "rconst", "sgconst", "wgt33", "qbasis", "basis2", "ones65"}
)
_DEV_CONSTS = {}


def _get_program(nchunk):
    if nchunk not in _PROG:
        _PROG[nchunk] = _build_program(nchunk)
    return _PROG[nchunk]


def _get_runner(nchunk):
    """Build (once) a jitted shard_map executable over the 8 cores.

    Mirrors concourse.bass2jax.run_bass_via_pjrt, but caches the traced/
    compiled callable so repeat kernel() calls skip retracing.
    """
    if nchunk in _RUNNER:
        return _RUNNER[nchunk]
    import jax
    import concourse.mybir as mybir
    from concourse import bass2jax
    from jax.experimental.shard_map import shard_map
    from jax.sharding import Mesh, PartitionSpec

    nc = _get_program(nchunk)
    bass2jax.install_neuronx_cc_hook()
    assert nc.dbg_addr is None
    partition_name = (
        nc.partition_id_tensor.name if nc.partition_id_tensor else None
    )
    in_names, out_names, out_avals, zero_outs = [], [], [], []
    for alloc in nc.m.functions[0].allocations:
        if not isinstance(alloc, mybir.MemoryLocationSet):
            continue
        name = alloc.memorylocations[0].name
        if alloc.kind == "ExternalInput":
            if name != partition_name:
                in_names.append(name)
        elif alloc.kind == "ExternalOutput":
            out_names.append(name)
            shape = tuple(alloc.tensor_shape)
            dtype = mybir.dt.np(alloc.dtype)
            out_avals.append(jax.core.ShapedArray(shape, dtype))
            zero_outs.append(np.zeros(shape, dtype))
    n_params = len(in_names)
    n_outs = len(out_avals)
    in_names_all = in_names + out_names
    if partition_name is not None:
        in_names_all.append(partition_name)
    donate = tuple(range(n_params, n_params + n_outs))

    def _body(*args):
        operands = list(args)
        if partition_name is not None:
            operands.append(bass2jax.partition_id_tensor())
        outs = bass2jax._bass_exec_p.bind(
            *operands,
            out_avals=tuple(out_avals),
            in_names=tuple(in_names_all),
            out_names=tuple(out_names),
            lowering_input_output_aliases=(),
            sim_require_finite=True,
            sim_require_nnan=True,
            nc=nc,
        )
        return tuple(outs)

    devices = jax.devices()[:NCORE]
    mesh = Mesh(np.asarray(devices), ("core",))
    sharded = jax.jit(
        shard_map(
            _body,
            mesh=mesh,
            in_specs=(PartitionSpec("core"),) * (n_params + n_outs),
            out_specs=(PartitionSpec("core"),) * n_outs,
            check_rep=False,
        ),
        donate_argnums=donate,
        keep_unused=True,
    )
    zero_concats = [
        np.zeros((NCORE * z.shape[0], *z.shape[1:]), z.dtype) for z in zero_outs
    ]
    info = (sharded, in_names, out_names, out_avals, zero_concats, mesh)
    _RUNNER[nchunk] = info
    return info


def _run(in_maps, nchunk):
    import jax
    from jax.sharding import NamedSharding, PartitionSpec

    sharded, in_names, out_names, out_avals, zero_concats, mesh = _get_runner(
        nchunk
    )
    args = []
    for name in in_names:
        if name in _CONST_NAMES:
            # constants live on device across calls: zero per-call transfer
            if name not in _DEV_CONSTS:
                concat = np.concatenate(
                    [in_maps[c][name] for c in range(NCORE)], axis=0
                )
                arr = jax.device_put(
                    concat, NamedSharding(mesh, PartitionSpec("core"))
                )
                arr.block_until_ready()
                _DEV_CONSTS[name] = arr
            args.append(_DEV_CONSTS[name])
        else:
            args.append(
                np.concatenate([in_maps[c][name] for c in range(NCORE)], axis=0)
            )
    out_arrs = sharded(*args, *[z.copy() for z in zero_concats])
    outs = {
        name: np.asarray(out_arrs[i]).reshape(NCORE, *out_avals[i].shape)
        for i, name in enumerate(out_names)
    }
    return [outs["out"][c] for c in range(NCORE)]


def kernel(angle_matrix, noise):
    nchunk = WPC // CB
    in_maps = _host_inputs(angle_matrix, noise, nchunk)
    core_outs = _run(in_maps, nchunk)
    return _finalize(core_outs)


# revision 19
# speedup vs baseline: 1.0237x; 1.0237x over previous
"""Trainium2 Bass kernel for nn_BatchFFTMA: 9216 independent 65x65 FFT-MA sims.

Math (validated against the jax reference; see v1 docstring for the
derivation of the phase folds):
  For each window w (patch p = noise[r0:r0+65, c0:c0+65], angle theta):
    Wf' = Cpt^T p Cpt   with Cpt = F*diag((-1)^k)
    E   = Re(Cq^T R Cq) with Cq = F*diag(e^{-2pi i 33 k/65})
    R   = exp(-sqrt(q)), q = alpha*x_r^2 + beta*x_c^2 + gamma*x_r*x_c
    gp  = sqrt(relu(E+1e-8)), gn = sqrt(relu(-(E+1e-8)))
    Xr  = Wf'_r*gp*SGP - Wf'_i*gn*SGN   (SGP/SGN: +-1 branch-cut fields)
    VC = sum(Xr); X00 = Xr[0,0]; S = sum((Wf'_r^2+Wf'_i^2)*|E+1e-8|)
    out_w = ((VC-X00)/N^2) / (sqrt((S-X00^2)/(N^2(N^2-1))) + 1e-6)

v2 compute structure (per core, 1152 windows, CB=18 windows/chunk):
  stage1 per window (lhsT = data):  o1p = P^T [Cr33|Ci33] (bf16),
                                    o1r = R^T [Qr33|Qi33] (f32)
  stage2 batched (lhsT = consts):   wfT_re/im = Cr^T o1pA -/+ ..., (bf16)
                                    E^T = Qr^T o1rA - Qi^T o1rB   (f32r)
  -> spectra come out transposed; all downstream sums are transpose-
  invariant. The spectral chain runs on halved columns k1=0..32 with x2
  weights folded into the SGP/SGN constants: the fields are Hermitian-even
  under joint index negation except on the k1+k2=65 line, where the E<0
  branch makes Xr odd (net zero) -> SGN weights are zeroed there.
  sqrt/exp phases batch over SC=4 chunks so the ACT table set switches only
  twice per superchunk (no table fits both sqrt and exp).

I/O strategy (axon tunnel: ~50-80ms fixed RTT + ~11ms/MB, so payload rules):
  constants committed to device once at runner build; per-call payload per
  core = noise rows [76,160] bf16 (24KB) + alpha/beta/gamma [3,1152] f32
  (14KB); final normalize on device, output [1,1152] f32 per core.
"""
import numpy as np
import ml_dtypes

H, W, D = 96, 96, 32
N = 65
N2 = N * N
A_, B_ = 15.0, 3.0
NCORE = 8
WPC = H * W // NCORE      # 1152 windows per core
RPC = H // NCORE          # 12 output rows per core
CB = 18                   # windows per chunk
GRP = 6                   # windows per stage1 matmul/PSUM group
NGRP = CB // GRP
BW = 9                    # windows per stage2 block
NH = 34                   # halved spectral columns (even, incl. both of the 32/33 mirror pair)
SC = 4                    # chunks per superchunk (ACT table batching)

_bf16 = ml_dtypes.bfloat16


def _host_constants():
    k = np.arange(N)
    F = np.exp(-2j * np.pi * np.outer(k, k) / N)
    Cpt = F * ((-1.0) ** k)[None, :]
    Cq = F * np.exp(-2j * np.pi * k * 33 / N)[None, :]
    Cr = Cpt.real.astype(np.float32)
    Ci = Cpt.imag.astype(np.float32)
    Qr = Cq.real.astype(np.float32)
    Qi = Cq.imag.astype(np.float32)
    pconst = np.concatenate(
        [Cr[:, :NH], Ci[:, :NH], Cr, Ci, -Ci], axis=1
    ).astype(_bf16)                                                # [65, 261]
    rconst = np.concatenate(
        [Qr[:, :NH], Qi[:, :NH], Qr, -Qi], axis=1
    ).astype(np.float32)                                           # [65, 196]

    k1, k2 = np.meshgrid(k, k, indexing="ij")
    ksum = k1 + k2
    m = ksum % N
    extra = (-1.0) ** (ksum // N)
    SGP = extra * np.where(m <= 32, 1.0, -1.0)
    SGN = extra * np.where(m == 0, 1.0, -1.0)
    wgt = np.ones(NH); wgt[1:32] = 2.0   # n=0,32,33 -> 1; n=1..31 -> 2
    SGP_h = SGP[:, :NH] * wgt[None, :]
    SGN_h = SGN[:, :NH] * wgt[None, :]
    for n in range(1, 32):
        SGN_h[(N - n) % N, n] = 0.0   # k1+k2=65 line: odd, net zero (mirror absent)
    sgp_t = np.tile(SGP_h, (1, CB)).astype(_bf16)   # [65, NH*CB]
    sgn_t = np.tile(SGN_h, (1, CB)).astype(_bf16)
    sgconst = np.concatenate([sgp_t, sgn_t], axis=1)  # [65, 2*NH*CB]
    wgt33 = np.tile(wgt[None, :], (N, 1)).astype(_bf16)  # [65, 33]

    x = np.linspace(-D, D, N, dtype=np.float32)
    x2 = x * x
    ones = np.ones(N, np.float32)
    # q[r, (w,c)] = x2[r]*rq[0] + 1*rq[1] + x[r]*rq[2]
    # rq[h, (w,c)] = abg[h, w] * basis2[h, c]
    qbasis = np.stack([x2, ones, x]).astype(np.float32)   # [3, 65]
    basis2 = np.stack([ones, x2, x]).astype(np.float32)   # [3, 65]
    ones65 = np.ones((N, 1), np.float32)
    return pconst, rconst, sgconst, wgt33, qbasis, basis2, ones65


def _build_program(nchunk):
    import concourse.bacc as bacc
    import concourse.mybir as mybir
    from concourse.tile import TileContext

    f32 = mybir.dt.float32
    f32r = mybir.dt.float32r
    f16 = mybir.dt.float16
    bf16 = mybir.dt.bfloat16
    AF = mybir.ActivationFunctionType
    ALU = mybir.AluOpType
    AX = mybir.AxisListType

    nwin = nchunk * CB
    assert nchunk % SC == 0

    nc = bacc.Bacc()
    nz_in = nc.declare_dram_parameter("nz", [RPC + N - 1, 160], bf16, isOutput=False)
    abg_in = nc.declare_dram_parameter("abg", [3, nwin], f16, isOutput=False)
    W2 = 2 * NH
    pconst_in = nc.declare_dram_parameter("pconst", [N, W2 + 195], bf16, isOutput=False)
    rconst_in = nc.declare_dram_parameter("rconst", [N, W2 + 130], f32, isOutput=False)
    sg_in = nc.declare_dram_parameter("sgconst", [N, 2 * NH * CB], bf16, isOutput=False)
    wgt_in = nc.declare_dram_parameter("wgt33", [N, NH], bf16, isOutput=False)
    qbasis_in = nc.declare_dram_parameter("qbasis", [3, N], f32, isOutput=False)
    basis2_in = nc.declare_dram_parameter("basis2", [3, N], f32, isOutput=False)
    ones_in = nc.declare_dram_parameter("ones65", [N, 1], f32, isOutput=False)
    out_d = nc.declare_dram_parameter("out", [1, nwin], f32, isOutput=True)

    with TileContext(nc) as tc:
        with (
            tc.tile_pool(name="const", bufs=1) as cpool,
            tc.tile_pool(name="qsb", bufs=2) as qpool,
            tc.tile_pool(name="rqsb", bufs=SC + 1) as rqpool,
            tc.tile_pool(name="rsb", bufs=2) as rpool,
            tc.tile_pool(name="o1sb", bufs=2) as o1pool,
            tc.tile_pool(name="wfsb", bufs=2) as wfpool,
            tc.tile_pool(name="spec", bufs=2) as spool,
            tc.tile_pool(name="psq", bufs=2, space="PSUM") as pq_,
            tc.tile_pool(name="ps1p", bufs=1, space="PSUM") as pp1,
            tc.tile_pool(name="ps1r", bufs=1, space="PSUM") as pr1,
            tc.tile_pool(name="psw2", bufs=1, space="PSUM") as pw2,
            tc.tile_pool(name="pse2", bufs=2, space="PSUM") as pe2,
        ):
            pc = cpool.tile([N, W2 + 195], bf16)
            nc.sync.dma_start(out=pc[:], in_=pconst_in[:])
            rc = cpool.tile([N, W2 + 130], f32)
            nc.sync.dma_start(out=rc[:], in_=rconst_in[:])
            sg = cpool.tile([N, 2 * NH * CB], bf16)
            nc.sync.dma_start(out=sg[:], in_=sg_in[:])
            wgt = cpool.tile([N, NH], bf16)
            nc.sync.dma_start(out=wgt[:], in_=wgt_in[:])
            qbasis = cpool.tile([3, N], f32)
            nc.sync.dma_start(out=qbasis[:], in_=qbasis_in[:])
            basis2 = cpool.tile([3, N], f32)
            nc.sync.dma_start(out=basis2[:], in_=basis2_in[:])
            ones65 = cpool.tile([N, 1], f32)
            nc.sync.dma_start(out=ones65[:], in_=ones_in[:])
            abg = cpool.tile([3, nwin], f16)
            nc.sync.dma_start(out=abg[:], in_=abg_in[:])
            # f32r copy of [Qr | -Qi] for the 1-cycle stage2 matmuls (the
            # verifier requires f32r operands to be produced as f32r)
            rcr = cpool.tile([N, 130], f32r, tag="rcr")
            nc.vector.tensor_copy(rcr[:], rc[:, W2:W2 + 130])

            strips = cpool.tile([N, RPC * 160], bf16)
            for i in range(RPC):
                nc.sync.dma_start(
                    out=strips[:, i * 160:(i + 1) * 160], in_=nz_in[i:i + N, :]
                )

            # clock warm-up: tiny reads so ACT/DVE observe each setup DMA
            # queue once, capping per-instruction sync waits downstream.
            warm_s = cpool.tile([1, 9], f32, tag="warm_s")
            warm_v = cpool.tile([1, 9], f32, tag="warm_v")
            for idx, src in enumerate(
                (strips, pc, rc, sg, wgt, qbasis, basis2, ones65, abg)
            ):
                nc.scalar.copy(warm_s[0:1, idx:idx + 1], src[0:1, 0:1])
                nc.vector.tensor_copy(warm_v[0:1, idx:idx + 1], src[0:1, 0:1])

            epsp = cpool.tile([N, 1], f32, tag="epsp")
            nc.gpsimd.memset(epsp[:], 1e-8)
            epsn = cpool.tile([N, 1], f32, tag="epsn")
            nc.gpsimd.memset(epsn[:], -1e-8)

            vcs = cpool.tile([N, nwin], f32)
            ss = cpool.tile([N, nwin], f32)
            mus = cpool.tile([1, nwin], f32)

            CWID = CB * N           # 1170 cols per chunk
            for sc0 in range(0, nchunk, SC):
                # ---- phase A: q fields + sqrt (ACT sqrt-table), SC-wide ----
                s_sb = qpool.tile([N, SC * CWID], f32, tag="s_sb")
                for j in range(SC):
                    cw0 = (sc0 + j) * CB
                    rq_sb = rqpool.tile([3, CB * N], f32, tag="rq")
                    rq3 = rq_sb[:].rearrange("p (w c) -> p w c", c=N)
                    abg_v = abg[:, cw0:cw0 + CB].unsqueeze(2).broadcast_to(
                        [3, CB, N]
                    )
                    b2_v = basis2[:].unsqueeze(1).broadcast_to([3, CB, N])
                    nc.vector.tensor_tensor(rq3, abg_v, b2_v, op=ALU.mult)
                    for h in range(3):
                        qps = pq_.tile([N, 390], f32, tag="qps")
                        nc.tensor.matmul(
                            qps[:], qbasis[:], rq_sb[:, h * 390:(h + 1) * 390],
                            start=True, stop=True,
                        )
                        nc.scalar.activation(
                            s_sb[:, j * CWID + h * 390:j * CWID + (h + 1) * 390],
                            qps[:], AF.Sqrt,
                        )
                # ---- phase B: ONE wide exp per superchunk (2 table loads) ----
                r_sb4 = rpool.tile([N, SC * CWID], f32, tag="r_sb")
                nc.scalar.activation(r_sb4[:], s_sb[:], AF.Exp, scale=-1.0)

                # ---- phase C: DFTs + spectral chain per chunk ----
                for j in range(SC):
                    ch = sc0 + j
                    cw0 = ch * CB
                    r_sb = r_sb4[:, j * CWID:(j + 1) * CWID]

                    o1p = o1pool.tile([N, CB * W2], bf16, tag="o1p")
                    o1r = o1pool.tile([N, CB * W2], f32r, tag="o1r")
                    for g in range(NGRP):
                        ps1p = pp1.tile([N, GRP * W2], f32, tag="ps1p")
                        for t in range(GRP):
                            w = cw0 + g * GRP + t
                            r0, c0 = divmod(w, 96)
                            lhs = strips[:, r0 * 160 + c0: r0 * 160 + c0 + N]
                            nc.tensor.matmul(
                                ps1p[:, t * W2:(t + 1) * W2], lhs, pc[:, 0:W2],
                                start=True, stop=True,
                            )
                        nc.scalar.copy(
                            o1p[:, g * GRP * W2:(g + 1) * GRP * W2], ps1p[:]
                        )
                        ps1r = pr1.tile([N, GRP * W2], f32, tag="ps1r")
                        for t in range(GRP):
                            wl = (g * GRP + t) * N
                            nc.tensor.matmul(
                                ps1r[:, t * W2:(t + 1) * W2],
                                r_sb[:, wl:wl + N], rc[:, 0:W2],
                                start=True, stop=True,
                            )
                        nc.vector.tensor_copy(
                            o1r[:, g * GRP * W2:(g + 1) * GRP * W2], ps1r[:]
                        )

                    # batched stage2 + spectral front-end, per 9-window block
                    wfA = wfpool.tile([N, CB * NH], bf16, tag="wfA")
                    wfB = wfpool.tile([N, CB * NH], bf16, tag="wfB")
                    rp = spool.tile([N, CB * NH], bf16, tag="rp")
                    rn = spool.tile([N, CB * NH], bf16, tag="rn")
                    azt = spool.tile([N, CB * NH], bf16, tag="azt")
                    for b in range(2):
                        o1p3 = o1p[:, b * BW * W2:(b + 1) * BW * W2].rearrange(
                            "p (w t) -> p w t", t=W2
                        )
                        o1r3 = o1r[:, b * BW * W2:(b + 1) * BW * W2].rearrange(
                            "p (w t) -> p w t", t=W2
                        )
                        sl = slice(b * BW * NH, (b + 1) * BW * NH)

                        wre = pw2.tile([N, BW * NH], f32, tag="wre")
                        nc.tensor.matmul(
                            wre[:], pc[:, W2:W2 + 65], o1p3[:, :, 0:NH],
                            start=True, stop=False,
                        )
                        nc.tensor.matmul(
                            wre[:], pc[:, W2 + 130:W2 + 195], o1p3[:, :, NH:W2],
                            start=False, stop=True,
                        )
                        nc.scalar.copy(wfA[:, sl], wre[:])
                        wim = pw2.tile([N, BW * NH], f32, tag="wim")
                        nc.tensor.matmul(
                            wim[:], pc[:, W2 + 65:W2 + 130], o1p3[:, :, 0:NH],
                            start=True, stop=False,
                        )
                        nc.tensor.matmul(
                            wim[:], pc[:, W2:W2 + 65], o1p3[:, :, NH:W2],
                            start=False, stop=True,
                        )
                        nc.scalar.copy(wfB[:, sl], wim[:])

                        e2 = pe2.tile([N, BW * NH], f32, tag="e2")
                        nc.tensor.matmul(
                            e2[:], rcr[:, 0:65], o1r3[:, :, 0:NH],
                            start=True, stop=False,
                        )
                        nc.tensor.matmul(
                            e2[:], rcr[:, 65:130], o1r3[:, :, NH:W2],
                            start=False, stop=True,
                        )
                        # rp = relu(E+eps), rn = relu(-(E+eps)), az = |E+eps|
                        nc.scalar.activation(rp[:, sl], e2[:], AF.Relu, bias=epsp[:])
                        nc.scalar.activation(
                            rn[:, sl], e2[:], AF.Relu, scale=-1.0, bias=epsn[:]
                        )

                    FD = CB * NH
                    # az = |E+eps| = relu(E+eps) + relu(-(E+eps))
                    nc.vector.tensor_add(azt[:], rp[:], rn[:])
                    azw = spool.tile([N, FD], bf16, tag="azw")
                    az3o = azw[:].rearrange("p (w c) -> p w c", c=NH)
                    az3i = azt[:].rearrange("p (w c) -> p w c", c=NH)
                    wgt_v = wgt[:].unsqueeze(1).broadcast_to([N, CB, NH])
                    nc.vector.tensor_tensor(az3o, az3i, wgt_v, op=ALU.mult)

                    gp = spool.tile([N, FD], bf16, tag="gp")
                    nc.scalar.activation(gp[:], rp[:], AF.Sqrt)
                    gn = spool.tile([N, FD], bf16, tag="gn")
                    nc.scalar.activation(gn[:], rn[:], AF.Sqrt)
                    gps = spool.tile([N, FD], bf16, tag="gps")
                    nc.vector.tensor_mul(gps[:], gp[:], sg[:, 0:FD])
                    gns = spool.tile([N, FD], bf16, tag="gns")
                    nc.vector.tensor_mul(gns[:], gn[:], sg[:, FD:2 * FD])
                    a_t = spool.tile([N, FD], bf16, tag="a_t")
                    nc.vector.tensor_mul(a_t[:], wfA[:], gps[:])
                    b_t = spool.tile([N, FD], bf16, tag="b_t")
                    nc.vector.tensor_mul(b_t[:], wfB[:], gns[:])
                    xr = spool.tile([N, FD], bf16, tag="xr")
                    nc.vector.tensor_sub(xr[:], a_t[:], b_t[:])
                    wr2 = spool.tile([N, FD], bf16, tag="wr2")
                    nc.vector.tensor_mul(wr2[:], wfA[:], wfA[:])
                    wi2 = spool.tile([N, FD], bf16, tag="wi2")
                    nc.vector.tensor_mul(wi2[:], wfB[:], wfB[:])
                    w2s = spool.tile([N, FD], bf16, tag="w2s")
                    nc.vector.tensor_add(w2s[:], wr2[:], wi2[:])
                    st = spool.tile([N, FD], bf16, tag="st")
                    nc.vector.tensor_mul(st[:], w2s[:], azw[:])

                    xr3 = xr[:].rearrange("p (w c) -> p w c", c=NH)
                    st3 = st[:].rearrange("p (w c) -> p w c", c=NH)
                    nc.vector.tensor_reduce(
                        vcs[:, cw0:cw0 + CB], xr3, axis=AX.X, op=ALU.add
                    )
                    nc.vector.tensor_reduce(
                        ss[:, cw0:cw0 + CB], st3, axis=AX.X, op=ALU.add
                    )
                    nc.vector.tensor_copy(mus[0:1, cw0:cw0 + CB], xr3[0:1, :, 0])

            # ---- on-device finalize: one scalar per window ----
            vc_row = cpool.tile([1, nwin], f32, tag="vc_row")
            s_row = cpool.tile([1, nwin], f32, tag="s_row")
            t1 = cpool.tile([1, nwin], f32, tag="t1")
            t2 = cpool.tile([1, nwin], f32, tag="t2")
            t3 = cpool.tile([1, nwin], f32, tag="t3")
            NB = 384
            for b0 in range(0, nwin, NB):
                bn = min(NB, nwin - b0)
                pv = pq_.tile([1, NB], f32, tag="qps")
                nc.tensor.matmul(
                    pv[:, 0:bn], ones65[:], vcs[:, b0:b0 + bn],
                    start=True, stop=True,
                )
                nc.vector.tensor_copy(vc_row[0:1, b0:b0 + bn], pv[:, 0:bn])
                pv2 = pq_.tile([1, NB], f32, tag="qps")
                nc.tensor.matmul(
                    pv2[:, 0:bn], ones65[:], ss[:, b0:b0 + bn],
                    start=True, stop=True,
                )
                nc.vector.tensor_copy(s_row[0:1, b0:b0 + bn], pv2[:, 0:bn])

            # out = ((VC-mu)/N2) / (sqrt(max(S-mu^2,0)/(N2*(N2-1))) + 1e-6)
            nc.vector.tensor_sub(t1[:], vc_row[:], mus[:])
            nc.vector.tensor_mul(t2[:], mus[:], mus[:])
            nc.vector.tensor_sub(t3[:], s_row[:], t2[:])
            nc.vector.tensor_scalar_max(t2[:], t3[:], 0.0)
            c2 = 1.0 / (float(N2) * (N2 - 1.0))
            nc.scalar.activation(t3[:], t2[:], AF.Sqrt, scale=c2)
            nc.vector.tensor_scalar_add(t2[:], t3[:], 1e-6)
            nc.vector.reciprocal(t3[:], t2[:])
            nc.vector.scalar_tensor_tensor(
                t2[:], t1[:], 1.0 / N2, t3[:], op0=ALU.mult, op1=ALU.mult
            )
            nc.sync.dma_start(out=out_d[:], in_=t2[:])
    if not nc.is_finalized():
        nc.finalize()
    return nc


_HOST_CONSTS = None


def _host_inputs(angle_matrix, noise, nchunk):
    """Per-core input maps. Core c owns output rows [12c, 12c+12)."""
    global _HOST_CONSTS
    if _HOST_CONSTS is None:
        _HOST_CONSTS = _host_constants()
    pconst, rconst, sgconst, wgt33, qbasis, basis2, ones65 = _HOST_CONSTS
    noise2d = np.asarray(noise, dtype=np.float32)[0, 0]
    nzb = noise2d.astype(_bf16)
    ang = np.asarray(angle_matrix, dtype=np.float32).reshape(-1).astype(np.float64)
    c = np.cos(ang); s = np.sin(ang)
    alpha = (c * c / A_**2 + s * s / B_**2).astype(np.float32)
    beta = (s * s / A_**2 + c * c / B_**2).astype(np.float32)
    gamma = (2 * c * s * (1 / A_**2 - 1 / B_**2)).astype(np.float32)
    abg_all = np.stack([alpha, beta, gamma])  # [3, 9216]

    in_maps = []
    for core in range(NCORE):
        w0 = core * WPC
        r_base = core * RPC
        in_maps.append({
            "nz": np.ascontiguousarray(nzb[r_base:r_base + RPC + N - 1, :]),
            "abg": np.ascontiguousarray(abg_all[:, w0:w0 + WPC]).astype(np.float16),
            "pconst": pconst,
            "rconst": rconst,
            "sgconst": sgconst,
            "wgt33": wgt33,
            "qbasis": qbasis,
            "basis2": basis2,
            "ones65": ones65,
        })
    return in_maps


def _finalize(core_outs):
    """core_outs: list of [1, 1152] arrays -> [96, 96] output."""
    return np.concatenate(
        [arr.reshape(RPC, 96) for arr in core_outs], axis=0
    ).astype(np.float32)


_PROG = {}
_RUNNER = {}
_CONST_NAMES = frozenset(
    {"pconst", "rconst", "sgconst", "wgt33", "qbasis", "basis2", "ones65"}
)
_DEV_CONSTS = {}


def _get_program(nchunk):
    if nchunk not in _PROG:
        _PROG[nchunk] = _build_program(nchunk)
    return _PROG[nchunk]


def _get_runner(nchunk):
    """Build (once) a jitted shard_map executable over the 8 cores.

    Mirrors concourse.bass2jax.run_bass_via_pjrt, but caches the traced/
    compiled callable so repeat kernel() calls skip retracing.
    """
    if nchunk in _RUNNER:
        return _RUNNER[nchunk]
    import jax
    import concourse.mybir as mybir
    from concourse import bass2jax
    from jax.experimental.shard_map import shard_map
    from jax.sharding import Mesh, PartitionSpec

    nc = _get_program(nchunk)
    bass2jax.install_neuronx_cc_hook()
    assert nc.dbg_addr is None
    partition_name = (
        nc.partition_id_tensor.name if nc.partition_id_tensor else None
    )
    in_names, out_names, out_avals, zero_outs = [], [], [], []
    for alloc in nc.m.functions[0].allocations:
        if not isinstance(alloc, mybir.MemoryLocationSet):
            continue
        name = alloc.memorylocations[0].name
        if alloc.kind == "ExternalInput":
            if name != partition_name:
                in_names.append(name)
        elif alloc.kind == "ExternalOutput":
            out_names.append(name)
            shape = tuple(alloc.tensor_shape)
            dtype = mybir.dt.np(alloc.dtype)
            out_avals.append(jax.core.ShapedArray(shape, dtype))
            zero_outs.append(np.zeros(shape, dtype))
    n_params = len(in_names)
    n_outs = len(out_avals)
    in_names_all = in_names + out_names
    if partition_name is not None:
        in_names_all.append(partition_name)
    donate = tuple(range(n_params, n_params + n_outs))

    def _body(*args):
        operands = list(args)
        if partition_name is not None:
            operands.append(bass2jax.partition_id_tensor())
        outs = bass2jax._bass_exec_p.bind(
            *operands,
            out_avals=tuple(out_avals),
            in_names=tuple(in_names_all),
            out_names=tuple(out_names),
            lowering_input_output_aliases=(),
            sim_require_finite=True,
            sim_require_nnan=True,
            nc=nc,
        )
        return tuple(outs)

    devices = jax.devices()[:NCORE]
    mesh = Mesh(np.asarray(devices), ("core",))
    sharded = jax.jit(
        shard_map(
            _body,
            mesh=mesh,
            in_specs=(PartitionSpec("core"),) * (n_params + n_outs),
            out_specs=(PartitionSpec("core"),) * n_outs,
            check_rep=False,
        ),
        donate_argnums=donate,
        keep_unused=True,
    )
    zero_concats = [
        np.zeros((NCORE * z.shape[0], *z.shape[1:]), z.dtype) for z in zero_outs
    ]
    info = (sharded, in_names, out_names, out_avals, zero_concats, mesh)
    _RUNNER[nchunk] = info
    return info


def _run(in_maps, nchunk):
    import jax
    from jax.sharding import NamedSharding, PartitionSpec

    sharded, in_names, out_names, out_avals, zero_concats, mesh = _get_runner(
        nchunk
    )
    args = []
    for name in in_names:
        if name in _CONST_NAMES:
            # constants live on device across calls: zero per-call transfer
            if name not in _DEV_CONSTS:
                concat = np.concatenate(
                    [in_maps[c][name] for c in range(NCORE)], axis=0
                )
                arr = jax.device_put(
                    concat, NamedSharding(mesh, PartitionSpec("core"))
                )
                arr.block_until_ready()
                _DEV_CONSTS[name] = arr
            args.append(_DEV_CONSTS[name])
        else:
            args.append(
                np.concatenate([in_maps[c][name] for c in range(NCORE)], axis=0)
            )
    last_err = None
    for _attempt in range(2):
        try:
            out_arrs = sharded(*args, *[z.copy() for z in zero_concats])
            outs = {
                name: np.asarray(out_arrs[i]).reshape(
                    NCORE, *out_avals[i].shape
                )
                for i, name in enumerate(out_names)
            }
            return [outs["out"][c] for c in range(NCORE)]
        except Exception as e:  # transient tunnel/device blips: retry once
            last_err = e
    raise last_err


def kernel(angle_matrix, noise):
    nchunk = WPC // CB
    in_maps = _host_inputs(angle_matrix, noise, nchunk)
    core_outs = _run(in_maps, nchunk)
    return _finalize(core_outs)
